# revision 23
# baseline (speedup 1.0000x reference)
"""Trainium2 Bass kernel for nn_GATrAutoRegressorLoss.

Strategy (data-parallel over valid assignment elements, 8 cores):
  - The dominant cost is the assignment BCE over (T=32, N=500000) logits,
    but only ~half the (t, hit) pairs are valid (t < c(hit)).  The host
    packs exactly the valid elements (hit-major) into one flat stream,
    splits it evenly across the 8 cores as (128, NV=7872) fp8 tiles
    (padded with -100, which contributes exactly 0), so the device never
    touches masked elements and needs no masking machinery at all.
  - Per chunk the device computes u = exp(x) on the Scalar engine (ACT,
    1 elem/cycle/partition regardless of dtype), then folds the softplus
    sum  sum ln(1+u)  through a pairwise product tree on the Vector
    engine: v = u+1 via tensor_scalar, then v_L*v_R tensor_tensor
    multiplies (both run in the fast packed-bf16 DVE modes) down to 1/16
    of the columns; the second ACT pass ln(p16) is split in two so the
    last chunk's slice is tiny and the tail stays short.  accum_out
    gives free per-partition row sums.  scalar_tensor_tensor is avoided
    in the hot path (it has no DVE perf modes).
  - The pid/stop exponentials ride the main exp pass: xq starts with 384
    extra fp8 columns holding the [pid0..4, stopx] (T,B) planes, covered
    by chunk 0's exp instruction, so the small-loss logsumexp/softplus
    need no separate exp instruction and no small-plane DMA dependency.
    se-1 overwrites the pid4 exp slice (after the sum-reduce consumed
    it) so a single ln over [se-1 | exp(stopx)] yields
    [logsumexp | softplus(stopx)] in one pass.
  - Chunk widths are graded (small first chunk so the first exp starts
    right after the fixed NEFF preamble, small last chunk so the final
    tree+ln tail is short); ut is quad-buffered so ACT never waits for
    the Vector engine.
  - The "- x*z" BCE terms of both the assignment and stop losses, all
    denominators, the pid class gather, and the direction dot/inverse-
    norm planes are exact host-side numpy (index bookkeeping).
  - Exp and Ln are pinned to the one ACT function table that contains
    both (see _Bacc) so the Scalar engine loads its table exactly once.
  - Per-core partial sums are returned and combined on the host in
    float64.  Any valid elements beyond device capacity (or |x| > 4.5
    outliers, which also bounds the product tree far below overflow) are
    folded in exactly on the host; for the reference input distribution
    the overflow set is ~10 elements and the tree maxes out around 1e7.
"""

import numpy as np

import concourse.bacc as bacc
import concourse.mybir as mybir
from concourse.tile import TileContext
from concourse.bass_utils import run_bass_kernel_spmd

F32 = mybir.dt.float32
BF16 = mybir.dt.bfloat16
F8 = mybir.dt.float8e4
NP_F8 = mybir.dt.np(F8)
NP_BF16 = mybir.dt.np(BF16)

T, B, N, NPFO = 32, 256, 500000, 4096
L_DIR, L_MAG, L_PID, L_CHG, L_ASN, L_STP = 1.0, 1.0, 1.0, 0.5, 1.0, 0.5

N_CORES = 8
P = 128                   # partitions
NV = 7872                 # packed valid columns per core
CAP = N_CORES * P * NV    # device element capacity
PAD = -100.0              # exp -> 0 exactly, ln(1+0) = 0
XCLIP = 4.5               # |x| above this handled on host (tree stays tiny)
SW = 64                   # small-plane free width (T*B = 8192 = 128*64)
XPL = 6 * SW              # xq prefix cols: [pid0..4, stopx] planes
# (data_offset, data_width) per chunk over the data region of xq
# (xq[:, XPL:XPL+NV]); chunk 0's exp additionally covers the XPL prefix
_CHUNKS = [(0, 512), (512, 1728), (2240, 2240), (4480, 2240), (6720, 1152)]
assert sum(w for _, w in _CHUNKS) == NV and all(w % 16 == 0 for _, w in _CHUNKS)
WMAX = max(w for _, w in _CHUNKS) + XPL
N16 = NV // 16            # total p16 width
N16A = sum(w for _, w in _CHUNKS[:3]) // 16   # first ln slice (chunks 0-2)

# small-loss planes, each (T*B,) flattened to (128, 64), bf16.
# pp..gch adjacent (paired subtract), vv0/vv1 two copies of the mask.
_PLANES = ["dm", "dc", "dot", "rnnv", "valid"]
NPL = len(_PLANES)

_nc_cache = None
last_result = None


class _Bacc(bacc.Bacc):
    """Bacc whose ACT-table chooser binds Exp/Ln to the one json table that
    contains both (natural_log_exp_and_others), so the Scalar engine never
    reloads function tables between exp and ln passes."""

    def insert_act_table_loads(self):
        from concourse.hw_specs import get_activation_tables

        has_activation = any(
            isinstance(i, mybir.InstActivation)
            for b in self.main_func.blocks
            for i in b.instructions
        )
        if not has_activation:
            return
        AF = mybir.ActivationFunctionType
        tables = []
        for name, fns in get_activation_tables(self.m.arch).items():
            if name != "natural_log_exp_and_others":
                fns = set(fns) - {AF.Exp, AF.Ln}
            tables.append((name, set(fns)))
        import bass_rust as _bass_rust

        _bass_rust.insert_act_table_loads(self, tables)


def _emit_small_losses(nc, sml, smt, acc, ub):
    """(T,B) losses; sums land in acc[:, 4:9].

    ub[:, 0:320] = exp(pid planes), ub[:, 320:384] = exp(stopx).

    acc[4] = sum(cos*valid)             loss_dir  = (vcnt - a4)/vcnt
    acc[5] = sum((pp-gp)^2 * valid)     loss_mag  = a5/vcnt
    acc[6] = sum((pch-gch)^2 * valid)   loss_chg  = a6/vcnt
    acc[7] = sum(lse * valid)           loss_pid  = (a7 - pid_sel)/vcnt
    acc[8] = sum(softplus(stopx))       loss_stop = (a8 - host_xz)/(T*B)
    """
    AF = mybir.ActivationFunctionType
    OP = mybir.AluOpType
    PLI = {n: i for i, n in enumerate(_PLANES)}

    def reg(name, k=1):
        i = PLI[name]
        return smt[:, i * SW : (i + k) * SW]

    valid = reg("valid")
    # se = sum_k exp(pid_k); then se-1 overwrites the pid4 exp slice so a
    # single ln covers [se-1 | exp(stopx)] -> [ln(se) | softplus(stopx)]
    se = sml.tile([P, SW], F32, name="sse", tag="sse")
    nc.vector.tensor_reduce(
        out=se[:], in_=ub[:, 0 : 5 * SW].rearrange("p (k j) -> p j k", k=5),
        axis=mybir.AxisListType.X, op=OP.add,
    )
    nc.vector.tensor_scalar(
        out=ub[:, 4 * SW : 5 * SW], in0=se[:], scalar1=-1.0,
        scalar2=None, op0=OP.add,
    )
    LL = sml.tile([P, 2 * SW], F32, name="sLL", tag="sLL")
    nc.scalar.activation(
        out=LL[:], in_=ub[:, 4 * SW : 6 * SW], func=AF.Ln, bias=1.0,
    )  # LL = [ln(se) | softplus(stopx)]

    o = sml.tile([P, 4 * SW], F32, name="so", tag="so")
    # dir: sum(dot * rnn*valid)
    nc.vector.scalar_tensor_tensor(
        out=o[:, 0:SW], in0=reg("dot"), scalar=1.0, in1=reg("rnnv"),
        op0=OP.mult, op1=OP.mult, accum_out=acc[:, 4:5],
    )
    # mag/chg: host ships (pp-gp)*valid and (pch-gch)*valid; square and
    # double row-sum here
    ds = sml.tile([P, 2 * SW], BF16, name="sds", tag="sds")
    nc.vector.tensor_mul(ds[:], reg("dm", 2), reg("dm", 2))
    nc.vector.tensor_reduce(
        out=acc[:, 5:7], in_=ds[:].rearrange("p (k j) -> p k j", k=2),
        axis=mybir.AxisListType.X, op=OP.add,
    )
    # pid: sum(valid * lse)
    nc.vector.scalar_tensor_tensor(
        out=o[:, SW : 2 * SW], in0=LL[:, 0:SW], scalar=1.0, in1=valid,
        op0=OP.mult, op1=OP.mult, accum_out=acc[:, 7:8],
    )
    # stop: sum(softplus(stopx))
    nc.vector.tensor_scalar(
        out=o[:, 2 * SW : 3 * SW], in0=LL[:, SW : 2 * SW], scalar1=1.0,
        scalar2=0.0, op0=OP.mult, op1=OP.add, accum_out=acc[:, 8:9],
    )


def _gen():
    nc = _Bacc(None, target_bir_lowering=False, debug=True)
    xq = nc.dram_tensor("xq", [P, XPL + NV], F8, kind="ExternalInput")
    sm = nc.dram_tensor("sm", [P, NPL * SW], BF16, kind="ExternalInput")
    partials = nc.dram_tensor("partials", [P, 16], F32, kind="ExternalOutput")
    s16b = nc.dram_tensor("s16b", [P, N16 - N16A], BF16, kind="ExternalOutput")

    AF = mybir.ActivationFunctionType
    OP = mybir.AluOpType

    # per-chunk tree groups over the contiguous exp buffer
    groups = [(XPL + c0, w, ci) for ci, (c0, w) in enumerate(_CHUNKS)]

    with TileContext(nc) as tc:
        with (
            tc.tile_pool(name="cst", bufs=1) as cst,
            tc.tile_pool(name="io", bufs=5) as io,
            tc.tile_pool(name="wk", bufs=2) as wk,
            tc.tile_pool(name="sml", bufs=1) as sml,
        ):
            acc = cst.tile([P, 16], F32)
            p16 = cst.tile([P, N16], BF16)
            ub = cst.tile([P, XPL + NV], BF16)  # all exp outputs, contiguous

            # stage input DMAs: chunk 0 and (via the ACT hwdge queue, in
            # parallel) chunk 1 first so the exp chain starts earliest;
            # then small planes, chunk 2, and merged chunks 3+4 on sync
            # two parallel DMA queues: sync HWDGE carries chunks
            # 0/1/3/4, gpsimd SWDGE carries the small planes and chunk 2
            smt = sml.tile([P, NPL * SW], BF16)
            nc.gpsimd.dma_start(out=smt[:], in_=sm[:])
            xts = []
            for ci, (c0, w) in enumerate(_CHUNKS):
                we = w + (XPL if ci == 0 else 0)
                d0 = 0 if ci == 0 else XPL + c0
                xt = io.tile([P, WMAX], F8, tag=f"x8{ci}")
                nc.sync.dma_start(out=xt[:, :we], in_=xq[:, d0 : d0 + we])
                xts.append(xt)

            gi = 0
            o16 = 0
            for ci, (c0, w) in enumerate(_CHUNKS):
                we = w + (XPL if ci == 0 else 0)
                u0 = 0 if ci == 0 else XPL + c0   # exp dest offset in ub
                nc.scalar.activation(
                    out=ub[:, u0 : u0 + we], in_=xts[ci][:, :we],
                    func=AF.Exp,
                )
                while gi < len(groups) and groups[gi][2] == ci:
                    g0, gw, _ = groups[gi]
                    w2, w4, w8, w16 = gw // 2, gw // 4, gw // 8, gw // 16
                    vt = wk.tile([P, max(g[1] for g in groups)], BF16,
                                 tag="vt")
                    nc.vector.tensor_scalar(
                        out=vt[:, :gw], in0=ub[:, g0 : g0 + gw], scalar1=1.0,
                        scalar2=None, op0=OP.add,
                    )
                    p2 = wk.tile([P, WMAX // 2], BF16, tag="p2")
                    nc.vector.tensor_mul(p2[:, :w2], vt[:, :w2], vt[:, w2:gw])
                    p4 = wk.tile([P, WMAX // 4], BF16, tag="p4")
                    nc.vector.tensor_mul(p4[:, :w4], p2[:, :w4], p2[:, w4:w2])
                    p8 = wk.tile([P, WMAX // 8], BF16, tag="p8")
                    nc.gpsimd.tensor_mul(p8[:, :w8], p4[:, :w8], p4[:, w8:w4])
                    nc.gpsimd.tensor_mul(
                        p16[:, o16 : o16 + w16], p8[:, :w16], p8[:, w16:w8]
                    )
                    o16 += w16
                    gi += 1
                if ci == 2:
                    _emit_small_losses(nc, sml, smt, acc, ub)

            s16 = cst.tile([P, N16], BF16)
            nc.scalar.activation(
                out=s16[:, :N16A], in_=p16[:, :N16A], func=AF.Ln,
                accum_out=acc[:, 0:1],
            )
            nc.sync.dma_start(out=partials[:], in_=acc[:])
            nc.scalar.activation(
                out=s16[:, N16A:], in_=p16[:, N16A:], func=AF.Ln,
            )
            nc.gpsimd.dma_start(out=s16b[:], in_=s16[:, N16A:])
    nc.finalize()
    return nc


def _get_nc():
    global _nc_cache
    if _nc_cache is None:
        _nc_cache = _gen()
    return _nc_cache


def _cumcount(gb):
    n = gb.shape[0]
    order = np.argsort(gb, kind="stable")
    sb = gb[order]
    first = np.searchsorted(sb, sb, side="left")
    cum = np.arange(n) - first
    out = np.zeros(n, dtype=np.int64)
    out[order] = cum
    return out


def kernel(**inputs):
    pfo_momentum = np.asarray(inputs["pfo_momentum"], np.float32)
    pfo_p_mod = np.asarray(inputs["pfo_p_mod"], np.float32)
    pfo_pid = np.asarray(inputs["pfo_pid"], np.float32)
    pfo_charge = np.asarray(inputs["pfo_charge"], np.float32)
    al = np.asarray(inputs["assignments_logits"], np.float32).reshape(T, N)
    stop_logits = np.asarray(inputs["stop_logits"], np.float32)
    gt_momentum = np.asarray(inputs["gt_momentum"], np.float32)
    gt_p_mod = np.asarray(inputs["gt_p_mod"], np.float32)
    gt_pid = np.asarray(inputs["gt_pid"], np.float32)
    gt_charge = np.asarray(inputs["gt_charge"], np.float32)
    gt_batch = np.asarray(inputs["gt_batch"]).astype(np.int64)
    hit_to_pfo = np.asarray(inputs["hit_to_pfo"]).astype(np.int64)
    hit_batch = np.asarray(inputs["hit_batch"]).astype(np.int64)

    # ---- host index bookkeeping ----
    ppe = np.bincount(gt_batch, minlength=B)[:B]                  # (B,)
    cmin = np.minimum(ppe[hit_batch], T).astype(np.int64)         # (N,)
    w = hit_to_pfo < cmin                                         # (N,) bool
    assign_den = max(float(cmin.sum()), 1.0)

    # exact selection term: sum over valid hits of x[hit_to_pfo[h], h]
    sel_sum = float(al[hit_to_pfo, np.arange(N)][w].sum(dtype=np.float64))

    # ---- pack valid assignment logits (hit-major) ----
    alT = np.ascontiguousarray(al.T)                              # (N, T)
    maskT = np.arange(T, dtype=np.int64)[None, :] < cmin[:, None]
    flat = alT[maskT]                                             # (V,) f32
    spill = 0.0
    big = np.abs(flat) > XCLIP
    if big.any():
        bv = flat[big].astype(np.float64)
        spill += float(np.logaddexp(0.0, bv).sum())
        flat = np.where(big, np.float32(PAD), flat)
    if flat.shape[0] > CAP:
        rest = flat[CAP:].astype(np.float64)
        keep = rest > PAD + 1.0  # skip already-padded outliers
        spill += float(np.logaddexp(0.0, rest[keep]).sum())
        flat = flat[:CAP]
    arr = np.full(CAP, PAD, np.float32)
    arr[: flat.shape[0]] = flat
    xd = arr.reshape(N_CORES, NV, P)

    step_idx = _cumcount(gt_batch)
    keep = step_idx < T
    si, gb = step_idx[keep], gt_batch[keep]

    def scat(vals):
        out = np.zeros((T, B) + vals.shape[1:], np.float32)
        out[si, gb] = vals[keep]
        return out

    gt_mom_tb = scat(gt_momentum)
    gt_pmod_tb = scat(gt_p_mod)
    gt_pid_tb = scat(gt_pid)
    gt_chg_tb = scat(gt_charge)

    steps = np.arange(T)[:, None]
    valid = (steps < ppe[None, :]).astype(np.float32)             # (T,B)
    vcnt = max(float(valid.sum()), 1.0)
    gt_stop = (steps >= ppe[None, :]).astype(np.float32)
    gt_cls = np.argmax(gt_pid_tb, axis=-1)                        # (T,B)
    # exact pid class-logit gather (host part of the cross entropy)
    pid_sel = float(
        (np.take_along_axis(pfo_pid, gt_cls[..., None], axis=-1)[..., 0]
         * valid).sum(dtype=np.float64)
    )
    # exact stop "x*z" term (host part of the stop BCE)
    host_xz = float(
        (stop_logits[..., 0] * gt_stop).sum(dtype=np.float64)
    )
    # direction dot & masked inverse-norm product planes
    dot = (pfo_momentum * gt_mom_tb).sum(axis=-1)                 # (T,B)
    na = np.maximum(np.linalg.norm(pfo_momentum, axis=-1), 1e-8)
    nb = np.maximum(np.linalg.norm(gt_mom_tb, axis=-1), 1e-8)
    rnnv = (valid / (na * nb)).astype(np.float32)

    def pack_plane(a):
        return np.ascontiguousarray(a.reshape(P, SW))

    # xq prefix: [pid0..4, stopx] planes as fp8 (ride chunk 0's exp)
    xpl = np.concatenate(
        [pack_plane(pfo_pid[..., k]) for k in range(5)]
        + [pack_plane(stop_logits[..., 0])],
        axis=1,
    )

    planes = {
        "dot": dot, "rnnv": rnnv,
        "dm": (pfo_p_mod[..., 0] - gt_pmod_tb[..., 0]) * valid,
        "dc": (pfo_charge[..., 0] - gt_chg_tb[..., 0]) * valid,
        "valid": valid,
    }
    sm = np.concatenate(
        [pack_plane(planes[n]) for n in _PLANES], axis=1
    ).astype(NP_BF16)

    in_maps = []
    for c in range(N_CORES):
        xfull = np.concatenate([xpl, xd[c].T], axis=1)
        in_maps.append(
            {"xq": np.ascontiguousarray(xfull).astype(NP_F8), "sm": sm}
        )

    nc = _get_nc()
    res = run_bass_kernel_spmd(nc, in_maps, core_ids=list(range(N_CORES)))
    global last_result
    last_result = res

    # ---- host combine (float64) ----
    A_sum = 0.0
    for c in range(N_CORES):
        pr = res.results[c]["partials"].astype(np.float64)
        A_sum += pr[:, 0].sum()
        A_sum += res.results[c]["s16b"].astype(np.float64).sum()
    loss_assign = (A_sum + spill - sel_sum) / assign_den

    pr0 = res.results[0]["partials"].astype(np.float64)
    a = pr0.sum(axis=0)
    loss_dir = (vcnt - a[4]) / vcnt
    loss_mag = a[5] / vcnt
    loss_chg = a[6] / vcnt
    loss_pid = (a[7] - pid_sel) / vcnt
    loss_stop = (a[8] - host_xz) / (T * B)

    total = (L_DIR * loss_dir + L_MAG * loss_mag + L_PID * loss_pid
             + L_CHG * loss_chg + L_ASN * loss_assign + L_STP * loss_stop)
    f = np.float32
    return (f(total), f(loss_dir), f(loss_mag), f(loss_pid), f(loss_chg),
            f(loss_assign), f(loss_stop))


# revision 24
# speedup vs baseline: 1.0821x; 1.0821x over previous
"""Trainium2 Bass kernel for nn_GATrAutoRegressorLoss.

Strategy (data-parallel over valid assignment elements, 8 cores):
  - The dominant cost is the assignment BCE over (T=32, N=500000) logits,
    but only ~half the (t, hit) pairs are valid (t < c(hit)).  The host
    packs exactly the valid elements (hit-major) into one flat stream,
    splits it evenly across the 8 cores as (128, NV=7872) fp8 tiles
    (padded with -100, which contributes exactly 0), so the device never
    touches masked elements and needs no masking machinery at all.
  - Per chunk the device computes u = exp(x) on the Scalar engine (ACT,
    1 elem/cycle/partition regardless of dtype), then folds the softplus
    sum  sum ln(1+u)  through a pairwise product tree on the Vector
    engine: v = u+1 via tensor_scalar, then v_L*v_R tensor_tensor
    multiplies (both run in the fast packed-bf16 DVE modes) down to 1/16
    of the columns; the second ACT pass ln(p16) is split in two so the
    last chunk's slice is tiny and the tail stays short.  accum_out
    gives free per-partition row sums.  scalar_tensor_tensor is avoided
    in the hot path (it has no DVE perf modes).
  - The pid/stop exponentials ride the main exp pass: xq starts with 384
    extra fp8 columns holding the [pid0..4, stopx] (T,B) planes, covered
    by chunk 0's exp instruction, so the small-loss logsumexp/softplus
    need no separate exp instruction and no small-plane DMA dependency.
    se-1 overwrites the pid4 exp slice (after the sum-reduce consumed
    it) so a single ln over [se-1 | exp(stopx)] yields
    [logsumexp | softplus(stopx)] in one pass.
  - Chunk widths are graded (small first chunk so the first exp starts
    right after the fixed NEFF preamble, small last chunk so the final
    tree+ln tail is short); ut is quad-buffered so ACT never waits for
    the Vector engine.
  - The "- x*z" BCE terms of both the assignment and stop losses, all
    denominators, the pid class gather, and the direction dot/inverse-
    norm planes are exact host-side numpy (index bookkeeping).
  - Exp and Ln are pinned to the one ACT function table that contains
    both (see _Bacc) so the Scalar engine loads its table exactly once.
  - Per-core partial sums are returned and combined on the host in
    float64.  Any valid elements beyond device capacity (or |x| > 4.5
    outliers, which also bounds the product tree far below overflow) are
    folded in exactly on the host; for the reference input distribution
    the overflow set is ~10 elements and the tree maxes out around 1e7.
"""

import numpy as np

import concourse.bacc as bacc
import concourse.mybir as mybir
from concourse.tile import TileContext
from concourse.bass_utils import run_bass_kernel_spmd

F32 = mybir.dt.float32
BF16 = mybir.dt.bfloat16
F8 = mybir.dt.float8e4
NP_F8 = mybir.dt.np(F8)
NP_BF16 = mybir.dt.np(BF16)

T, B, N, NPFO = 32, 256, 500000, 4096
L_DIR, L_MAG, L_PID, L_CHG, L_ASN, L_STP = 1.0, 1.0, 1.0, 0.5, 1.0, 0.5

N_CORES = 8
P = 128                   # partitions
NV = 7872                 # packed valid columns per core
CAP = N_CORES * P * NV    # device element capacity
PAD = -100.0              # exp -> 0 exactly, ln(1+0) = 0
XCLIP = 4.5               # |x| above this handled on host (tree stays tiny)
SW = 64                   # small-plane free width (T*B = 8192 = 128*64)
XPL = 6 * SW              # xq prefix cols: [pid0..4, stopx] planes
# (data_offset, data_width) per chunk over the data region of xq
# (xq[:, XPL:XPL+NV]); chunk 0's exp additionally covers the XPL prefix
_CHUNKS = [(0, 512), (512, 1728), (2240, 2240), (4480, 2240), (6720, 1152)]
assert sum(w for _, w in _CHUNKS) == NV and all(w % 16 == 0 for _, w in _CHUNKS)
WMAX = max(w for _, w in _CHUNKS) + XPL
N16 = NV // 16            # total p16 width
N16A = sum(w for _, w in _CHUNKS[:3]) // 16   # first ln slice (chunks 0-2)

# small-loss planes, each (T*B,) flattened to (128, 64), bf16.
# pp..gch adjacent (paired subtract), vv0/vv1 two copies of the mask.
_PLANES = ["dm", "dc", "dot", "rnnv", "valid"]
NPL = len(_PLANES)

_nc_cache = None
last_result = None


class _Bacc(bacc.Bacc):
    """Bacc whose ACT-table chooser binds Exp/Ln to the one json table that
    contains both (natural_log_exp_and_others), so the Scalar engine never
    reloads function tables between exp and ln passes."""

    def insert_act_table_loads(self):
        from concourse.hw_specs import get_activation_tables

        has_activation = any(
            isinstance(i, mybir.InstActivation)
            for b in self.main_func.blocks
            for i in b.instructions
        )
        if not has_activation:
            return
        AF = mybir.ActivationFunctionType
        tables = []
        for name, fns in get_activation_tables(self.m.arch).items():
            if name != "natural_log_exp_and_others":
                fns = set(fns) - {AF.Exp, AF.Ln}
            tables.append((name, set(fns)))
        import bass_rust as _bass_rust

        _bass_rust.insert_act_table_loads(self, tables)


def _emit_small_losses(nc, sml, smt, acc, ub):
    """(T,B) losses; sums land in acc[:, 4:9].

    ub[:, 0:320] = exp(pid planes), ub[:, 320:384] = exp(stopx).

    acc[4] = sum(cos*valid)             loss_dir  = (vcnt - a4)/vcnt
    acc[5] = sum((pp-gp)^2 * valid)     loss_mag  = a5/vcnt
    acc[6] = sum((pch-gch)^2 * valid)   loss_chg  = a6/vcnt
    acc[7] = sum(lse * valid)           loss_pid  = (a7 - pid_sel)/vcnt
    acc[8] = sum(softplus(stopx))       loss_stop = (a8 - host_xz)/(T*B)
    """
    AF = mybir.ActivationFunctionType
    OP = mybir.AluOpType
    PLI = {n: i for i, n in enumerate(_PLANES)}

    def reg(name, k=1):
        i = PLI[name]
        return smt[:, i * SW : (i + k) * SW]

    valid = reg("valid")
    # se = sum_k exp(pid_k); then se-1 overwrites the pid4 exp slice so a
    # single ln covers [se-1 | exp(stopx)] -> [ln(se) | softplus(stopx)]
    se = sml.tile([P, SW], F32, name="sse", tag="sse")
    nc.vector.tensor_reduce(
        out=se[:], in_=ub[:, 0 : 5 * SW].rearrange("p (k j) -> p j k", k=5),
        axis=mybir.AxisListType.X, op=OP.add,
    )
    nc.vector.tensor_scalar(
        out=ub[:, 4 * SW : 5 * SW], in0=se[:], scalar1=-1.0,
        scalar2=None, op0=OP.add,
    )
    LL = sml.tile([P, 2 * SW], F32, name="sLL", tag="sLL")
    nc.scalar.activation(
        out=LL[:], in_=ub[:, 4 * SW : 6 * SW], func=AF.Ln, bias=1.0,
    )  # LL = [ln(se) | softplus(stopx)]

    o = sml.tile([P, 4 * SW], F32, name="so", tag="so")
    # dir: sum(dot * rnn*valid)
    nc.vector.scalar_tensor_tensor(
        out=o[:, 0:SW], in0=reg("dot"), scalar=1.0, in1=reg("rnnv"),
        op0=OP.mult, op1=OP.mult, accum_out=acc[:, 4:5],
    )
    # mag/chg: host ships (pp-gp)*valid and (pch-gch)*valid; square and
    # double row-sum here
    ds = sml.tile([P, 2 * SW], BF16, name="sds", tag="sds")
    nc.vector.tensor_mul(ds[:], reg("dm", 2), reg("dm", 2))
    nc.vector.tensor_reduce(
        out=acc[:, 5:7], in_=ds[:].rearrange("p (k j) -> p k j", k=2),
        axis=mybir.AxisListType.X, op=OP.add,
    )
    # pid: sum(valid * lse)
    nc.vector.scalar_tensor_tensor(
        out=o[:, SW : 2 * SW], in0=LL[:, 0:SW], scalar=1.0, in1=valid,
        op0=OP.mult, op1=OP.mult, accum_out=acc[:, 7:8],
    )
    # stop: sum(softplus(stopx))
    nc.vector.tensor_scalar(
        out=o[:, 2 * SW : 3 * SW], in0=LL[:, SW : 2 * SW], scalar1=1.0,
        scalar2=0.0, op0=OP.mult, op1=OP.add, accum_out=acc[:, 8:9],
    )


def _gen():
    nc = _Bacc(None, target_bir_lowering=False, debug=True)
    xq = nc.dram_tensor("xq", [P, XPL + NV], F8, kind="ExternalInput")
    sm = nc.dram_tensor("sm", [P, NPL * SW], BF16, kind="ExternalInput")
    partials = nc.dram_tensor("partials", [P, 16], F32, kind="ExternalOutput")
    s16b = nc.dram_tensor("s16b", [P, N16 - N16A], BF16, kind="ExternalOutput")

    AF = mybir.ActivationFunctionType
    OP = mybir.AluOpType

    # per-chunk tree groups over the contiguous exp buffer
    groups = [(XPL + c0, w, ci) for ci, (c0, w) in enumerate(_CHUNKS)]

    with TileContext(nc) as tc:
        with (
            tc.tile_pool(name="cst", bufs=1) as cst,
            tc.tile_pool(name="io", bufs=5) as io,
            tc.tile_pool(name="wk", bufs=2) as wk,
            tc.tile_pool(name="sml", bufs=1) as sml,
        ):
            acc = cst.tile([P, 16], F32)
            p16 = cst.tile([P, N16], BF16)
            ub = cst.tile([P, XPL + NV], BF16)  # all exp outputs, contiguous

            # stage input DMAs: chunk 0 and (via the ACT hwdge queue, in
            # parallel) chunk 1 first so the exp chain starts earliest;
            # then small planes, chunk 2, and merged chunks 3+4 on sync
            # two parallel DMA queues: sync HWDGE carries chunks
            # 0/1/3/4, gpsimd SWDGE carries the small planes and chunk 2
            smt = sml.tile([P, NPL * SW], BF16)
            nc.gpsimd.dma_start(out=smt[:], in_=sm[:])
            xts = []
            for ci, (c0, w) in enumerate(_CHUNKS):
                we = w + (XPL if ci == 0 else 0)
                d0 = 0 if ci == 0 else XPL + c0
                xt = io.tile([P, WMAX], F8, tag=f"x8{ci}")
                nc.sync.dma_start(out=xt[:, :we], in_=xq[:, d0 : d0 + we])
                xts.append(xt)

            gi = 0
            o16 = 0
            for ci, (c0, w) in enumerate(_CHUNKS):
                we = w + (XPL if ci == 0 else 0)
                u0 = 0 if ci == 0 else XPL + c0   # exp dest offset in ub
                nc.scalar.activation(
                    out=ub[:, u0 : u0 + we], in_=xts[ci][:, :we],
                    func=AF.Exp,
                )
                while gi < len(groups) and groups[gi][2] == ci:
                    g0, gw, _ = groups[gi]
                    w2, w4, w8, w16 = gw // 2, gw // 4, gw // 8, gw // 16
                    vt = wk.tile([P, max(g[1] for g in groups)], BF16,
                                 tag="vt")
                    nc.vector.tensor_scalar(
                        out=vt[:, :gw], in0=ub[:, g0 : g0 + gw], scalar1=1.0,
                        scalar2=None, op0=OP.add,
                    )
                    p2 = wk.tile([P, WMAX // 2], BF16, tag="p2")
                    nc.vector.tensor_mul(p2[:, :w2], vt[:, :w2], vt[:, w2:gw])
                    p4 = wk.tile([P, WMAX // 4], BF16, tag="p4")
                    nc.vector.tensor_mul(p4[:, :w4], p2[:, :w4], p2[:, w4:w2])
                    p8 = wk.tile([P, WMAX // 8], BF16, tag="p8")
                    nc.vector.tensor_mul(p8[:, :w8], p4[:, :w8], p4[:, w8:w4])
                    nc.vector.tensor_mul(
                        p16[:, o16 : o16 + w16], p8[:, :w16], p8[:, w16:w8]
                    )
                    o16 += w16
                    gi += 1
                if ci == 2:
                    _emit_small_losses(nc, sml, smt, acc, ub)

            s16 = cst.tile([P, N16], BF16)
            nc.scalar.activation(
                out=s16[:, :N16A], in_=p16[:, :N16A], func=AF.Ln,
                accum_out=acc[:, 0:1],
            )
            nc.sync.dma_start(out=partials[:], in_=acc[:])
            nc.scalar.activation(
                out=s16[:, N16A:], in_=p16[:, N16A:], func=AF.Ln,
            )
            nc.gpsimd.dma_start(out=s16b[:], in_=s16[:, N16A:])
    nc.finalize()
    return nc


def _get_nc():
    global _nc_cache
    if _nc_cache is None:
        _nc_cache = _gen()
    return _nc_cache


def _cumcount(gb):
    n = gb.shape[0]
    order = np.argsort(gb, kind="stable")
    sb = gb[order]
    first = np.searchsorted(sb, sb, side="left")
    cum = np.arange(n) - first
    out = np.zeros(n, dtype=np.int64)
    out[order] = cum
    return out


def kernel(**inputs):
    pfo_momentum = np.asarray(inputs["pfo_momentum"], np.float32)
    pfo_p_mod = np.asarray(inputs["pfo_p_mod"], np.float32)
    pfo_pid = np.asarray(inputs["pfo_pid"], np.float32)
    pfo_charge = np.asarray(inputs["pfo_charge"], np.float32)
    al = np.asarray(inputs["assignments_logits"], np.float32).reshape(T, N)
    stop_logits = np.asarray(inputs["stop_logits"], np.float32)
    gt_momentum = np.asarray(inputs["gt_momentum"], np.float32)
    gt_p_mod = np.asarray(inputs["gt_p_mod"], np.float32)
    gt_pid = np.asarray(inputs["gt_pid"], np.float32)
    gt_charge = np.asarray(inputs["gt_charge"], np.float32)
    gt_batch = np.asarray(inputs["gt_batch"]).astype(np.int64)
    hit_to_pfo = np.asarray(inputs["hit_to_pfo"]).astype(np.int64)
    hit_batch = np.asarray(inputs["hit_batch"]).astype(np.int64)

    # ---- host index bookkeeping ----
    ppe = np.bincount(gt_batch, minlength=B)[:B]                  # (B,)
    cmin = np.minimum(ppe[hit_batch], T).astype(np.int64)         # (N,)
    w = hit_to_pfo < cmin                                         # (N,) bool
    assign_den = max(float(cmin.sum()), 1.0)

    # exact selection term: sum over valid hits of x[hit_to_pfo[h], h]
    sel_sum = float(al[hit_to_pfo, np.arange(N)][w].sum(dtype=np.float64))

    # ---- pack valid assignment logits (hit-major) ----
    alT = np.ascontiguousarray(al.T)                              # (N, T)
    maskT = np.arange(T, dtype=np.int64)[None, :] < cmin[:, None]
    flat = alT[maskT]                                             # (V,) f32
    spill = 0.0
    big = np.abs(flat) > XCLIP
    if big.any():
        bv = flat[big].astype(np.float64)
        spill += float(np.logaddexp(0.0, bv).sum())
        flat = np.where(big, np.float32(PAD), flat)
    if flat.shape[0] > CAP:
        rest = flat[CAP:].astype(np.float64)
        keep = rest > PAD + 1.0  # skip already-padded outliers
        spill += float(np.logaddexp(0.0, rest[keep]).sum())
        flat = flat[:CAP]
    arr = np.full(CAP, PAD, np.float32)
    arr[: flat.shape[0]] = flat
    xd = arr.reshape(N_CORES, NV, P)

    step_idx = _cumcount(gt_batch)
    keep = step_idx < T
    si, gb = step_idx[keep], gt_batch[keep]

    def scat(vals):
        out = np.zeros((T, B) + vals.shape[1:], np.float32)
        out[si, gb] = vals[keep]
        return out

    gt_mom_tb = scat(gt_momentum)
    gt_pmod_tb = scat(gt_p_mod)
    gt_pid_tb = scat(gt_pid)
    gt_chg_tb = scat(gt_charge)

    steps = np.arange(T)[:, None]
    valid = (steps < ppe[None, :]).astype(np.float32)             # (T,B)
    vcnt = max(float(valid.sum()), 1.0)
    gt_stop = (steps >= ppe[None, :]).astype(np.float32)
    gt_cls = np.argmax(gt_pid_tb, axis=-1)                        # (T,B)
    # exact pid class-logit gather (host part of the cross entropy)
    pid_sel = float(
        (np.take_along_axis(pfo_pid, gt_cls[..., None], axis=-1)[..., 0]
         * valid).sum(dtype=np.float64)
    )
    # exact stop "x*z" term (host part of the stop BCE)
    host_xz = float(
        (stop_logits[..., 0] * gt_stop).sum(dtype=np.float64)
    )
    # direction dot & masked inverse-norm product planes
    dot = (pfo_momentum * gt_mom_tb).sum(axis=-1)                 # (T,B)
    na = np.maximum(np.linalg.norm(pfo_momentum, axis=-1), 1e-8)
    nb = np.maximum(np.linalg.norm(gt_mom_tb, axis=-1), 1e-8)
    rnnv = (valid / (na * nb)).astype(np.float32)

    def pack_plane(a):
        return np.ascontiguousarray(a.reshape(P, SW))

    # xq prefix: [pid0..4, stopx] planes as fp8 (ride chunk 0's exp)
    xpl = np.concatenate(
        [pack_plane(pfo_pid[..., k]) for k in range(5)]
        + [pack_plane(stop_logits[..., 0])],
        axis=1,
    )

    planes = {
        "dot": dot, "rnnv": rnnv,
        "dm": (pfo_p_mod[..., 0] - gt_pmod_tb[..., 0]) * valid,
        "dc": (pfo_charge[..., 0] - gt_chg_tb[..., 0]) * valid,
        "valid": valid,
    }
    sm = np.concatenate(
        [pack_plane(planes[n]) for n in _PLANES], axis=1
    ).astype(NP_BF16)

    in_maps = []
    for c in range(N_CORES):
        xfull = np.concatenate([xpl, xd[c].T], axis=1)
        in_maps.append(
            {"xq": np.ascontiguousarray(xfull).astype(NP_F8), "sm": sm}
        )

    nc = _get_nc()
    res = run_bass_kernel_spmd(nc, in_maps, core_ids=list(range(N_CORES)))
    global last_result
    last_result = res

    # ---- host combine (float64) ----
    A_sum = 0.0
    for c in range(N_CORES):
        pr = res.results[c]["partials"].astype(np.float64)
        A_sum += pr[:, 0].sum()
        A_sum += res.results[c]["s16b"].astype(np.float64).sum()
    loss_assign = (A_sum + spill - sel_sum) / assign_den

    pr0 = res.results[0]["partials"].astype(np.float64)
    a = pr0.sum(axis=0)
    loss_dir = (vcnt - a[4]) / vcnt
    loss_mag = a[5] / vcnt
    loss_chg = a[6] / vcnt
    loss_pid = (a[7] - pid_sel) / vcnt
    loss_stop = (a[8] - host_xz) / (T * B)

    total = (L_DIR * loss_dir + L_MAG * loss_mag + L_PID * loss_pid
             + L_CHG * loss_chg + L_ASN * loss_assign + L_STP * loss_stop)
    f = np.float32
    return (f(total), f(loss_dir), f(loss_mag), f(loss_pid), f(loss_chg),
            f(loss_assign), f(loss_stop))
